# revision 62
# baseline (speedup 1.0000x reference)
"""GAT (3-layer, 4/4/1 heads) on 8 TRN2 NeuronCores.

Design (per core, SPMD one program):
- Nodes dealt to cores by degree (snake); within a core, nodes 2D-packed by
  (deg_A, deg_B) into 128-node blocks, blocks sorted by slot width desc.
- Blocks are packed into GROUPS with a uniform per-half slot count
  (TAg/TBg = max over member blocks): the whole group's A-half slots form
  one contiguous run (pads point at the poisoned pad row whose el slots are
  -30000, so they vanish in the softmax), so each (group, half) is ONE
  dma_gather and the softmax/aggregation chain runs group-wide with large
  contiguous access patterns.
- Gathers are issued prepare_only + trigger_dma: the Pool engine only pays
  descriptor *generation* (~0.34ns/desc), while the 16 SDMA engines drain
  rings asynchronously; 4 SWDGE queues round-robin.
- Per-edge records are fp16, exactly 256B: el = al.feat is linear in feat,
  so the record stores a permuted basis of feat where coordinate 31 of each
  head block holds el; the displaced feat coordinate is reconstructed after
  aggregation with one small weighted reduce (host bakes the permutation +
  al into the layer's M matrix and into the next layer's W).
- AllGather per layer into a Shared DRAM table.
- Edge phase per group: et = el_view(grid) + er (fused strided add) ->
  leaky -> per-(node,head) max over slots (A|B reduces + combine) -> exp ->
  denom -> alpha; grid *= alpha in place; binary tree of contiguous adds
  reduces slots per half, halves combined; group-wide recon/bias/relu;
  dense phase of layer L+1 (PE matmul) interleaved per block.
"""

import os
import numpy as np

import concourse.bacc as bacc
import concourse.bass as bass
import concourse.tile as tile
from concourse import mybir
from concourse.masks import make_identity

F32 = mybir.dt.float32
F16 = mybir.dt.float16
I16 = mybir.dt.int16

PAD_EL = -30000.0
SLOT_BUDGET = 48   # target slots (A+B) per group


# ---------------------------------------------------------------- host prep

def prep_host(src, dst, N, n_cores=8, blk=128):
    """Graph -> per-core grouped slot layout (owner-half A/B split)."""
    src = np.asarray(src); dst = np.asarray(dst)
    deg = np.bincount(dst, minlength=N)

    order = np.argsort(-deg, kind="stable")

    # window-balanced snake deal: within each 16-rank window (degree-similar
    # nodes), pick which 8 go to half A so every dst's in-degree splits
    # near-evenly between halves (shrinks per-block maxA+maxB slot counts).
    so = np.argsort(src, kind="stable")
    s_sorted = src[so]; d_sorted = dst[so]
    ostart = np.searchsorted(s_sorted, np.arange(N))
    oend = np.searchsorted(s_sorted, np.arange(N), side="right")
    Acores = [0, 1, 2, 3, 3, 2, 1, 0]
    Bcores = [4, 5, 6, 7, 7, 6, 5, 4]
    diff = np.zeros(N, np.float64)   # dA - dB so far per dst
    sign = np.zeros(N, np.float64)   # +1 assigned A, -1 assigned B
    owner = np.empty(N, np.int32)
    assert N % 16 == 0
    for _pass in range(3):
        for w in range(N // 16):
            nodes = order[w * 16:(w + 1) * 16]
            for v in nodes:
                if sign[v]:
                    diff[d_sorted[ostart[v]:oend[v]]] -= sign[v]
            pref = np.array([diff[d_sorted[ostart[v]:oend[v]]].sum() for v in nodes])
            sel = np.argsort(pref, kind="stable")   # lowest pref -> half A
            for i, j in enumerate(sel[:8]):
                v = nodes[j]; owner[v] = Acores[i]; sign[v] = 1.0
                diff[d_sorted[ostart[v]:oend[v]]] += 1
            for i, j in enumerate(sel[8:]):
                v = nodes[j]; owner[v] = Bcores[i]; sign[v] = -1.0
                diff[d_sorted[ostart[v]:oend[v]]] -= 1

    half_of_core = (np.arange(n_cores) >= n_cores // 2).astype(np.int32)
    src_half = half_of_core[owner[src]]
    degA = np.bincount(dst[src_half == 0], minlength=N)
    degB = np.bincount(dst[src_half == 1], minlength=N)

    core_nodes = [np.where(owner == c)[0] for c in range(n_cores)]
    npc_real = max(len(cn) for cn in core_nodes)
    assert all(len(cn) == npc_real for cn in core_nodes)
    nblk = (npc_real + blk) // blk          # >=1 pad row per core
    npc = nblk * blk

    layouts = []
    TA = np.zeros((n_cores, nblk), np.int32)
    TB = np.zeros((n_cores, nblk), np.int32)
    for c in range(n_cores):
        cn = core_nodes[c]
        key = np.maximum(degA[cn], degB[cn]) * 1000 + np.minimum(degA[cn], degB[cn])
        cn = cn[np.argsort(-key, kind="stable")]
        lay = np.full(npc, -1, np.int64)
        lay[:len(cn)] = cn
        for b in range(nblk):
            rowsb = lay[b * blk:(b + 1) * blk]
            real = rowsb[rowsb >= 0]
            TA[c, b] = degA[real].max() if len(real) else 0
            TB[c, b] = degB[real].max() if len(real) else 0
        bo = np.argsort(-(TA[c] + TB[c]), kind="stable")
        lay = np.concatenate([lay[b * blk:(b + 1) * blk] for b in bo])
        TA[c], TB[c] = TA[c][bo], TB[c][bo]
        assert (lay[npc_real:] < 0).all(), "pad rows must stay in the last block"
        layouts.append(lay)

    TAs = TA.max(axis=0)
    TBs = TB.max(axis=0)

    # ---- group formation (shared across cores) ----
    # the last few groups use a smaller slot budget so the per-layer serial
    # tail (last drains -> softmax -> dense -> allgather) stays short.
    groups = []  # (b0, b1, TAg, TBg)
    b = 0
    while b < nblk:
        budget = SLOT_BUDGET
        g = 1
        TAg, TBg = int(TAs[b]), int(TBs[b])
        while b + g < nblk:
            nTA = max(TAg, int(TAs[b + g]))
            nTB = max(TBg, int(TBs[b + g]))
            if (g + 1) * (nTA + nTB) > budget:
                break
            TAg, TBg = nTA, nTB
            g += 1
        groups.append((b, b + g, TAg, TBg))
        b += g

    soff = {}
    off = 0
    for gi, (b0, b1, TAg, TBg) in enumerate(groups):
        G = b1 - b0
        soff[(gi, 0)] = off; off += G * TAg
        soff[(gi, 1)] = off; off += G * TBg
    tot_slots = off

    half_rows = (n_cores // 2) * npc
    assert half_rows < 32768
    pi = np.full(N, -1, np.int64)
    for c in range(n_cores):
        rowsb = layouts[c]
        valid = rowsb >= 0
        pi[rowsb[valid]] = c * npc + np.where(valid)[0]
    assert (pi[np.unique(dst)] >= 0).all()

    eorder = np.argsort(dst, kind="stable")
    esrc = src[eorder]; edst = dst[eorder]
    estart = np.searchsorted(edst, np.arange(N))
    eend = np.searchsorted(edst, np.arange(N), side="right")

    idx_mega = np.zeros((n_cores, 128, 8 * tot_slots), np.int16)
    for c in range(n_cores):
        lay = layouts[c]
        for gi, (b0, b1, TAg, TBg) in enumerate(groups):
            G = b1 - b0
            for half, Tg in ((0, TAg), (1, TBg)):
                if Tg == 0:
                    continue
                o = soff[(gi, half)]
                S = G * Tg
                base = half * half_rows
                # t-major slot order (slot = t*G + g): every tree-reduce
                # level on device is then one flat contiguous DVE add.
                iv = np.full((128, S), half_rows - 1, np.int64)  # pad row
                for g in range(G):
                    rowsb = lay[(b0 + g) * blk:(b0 + g + 1) * blk]
                    for p in range(blk):
                        v = rowsb[p]
                        if v < 0:
                            continue
                        r = pi[esrc[estart[v]:eend[v]]]
                        mysrc = r[r >= half_rows] if half else r[r < half_rows]
                        k = len(mysrc)
                        assert k <= Tg
                        iv[p, g::G][:k] = mysrc - base
                assert (iv >= 0).all() and (iv < 32768).all()
                pos = np.arange(128 * S)
                wrapped = np.zeros((16, 8 * S), np.int16)
                wrapped[pos % 16, pos // 16] = iv[pos % 128, pos // 128].astype(np.int16)
                idx_mega[c, :, o * 8:(o + S) * 8] = np.tile(wrapped, (8, 1))

    return dict(
        n_cores=n_cores, blk=blk, nblk=nblk, npc=npc, n_real=npc_real,
        half_rows=half_rows, NROWS=n_cores * npc, tot_slots=tot_slots,
        soff=soff, groups=groups, TA=TAs, TB=TBs, layouts=layouts, pi=pi,
        idx_mega=idx_mega,
    )


# ------------------------------------------------------------ device build

def build_program(st, H=4, DH=32, D_IN=128, D_OUT=64, nq=4):
    n_cores, blk, nblk, npc = st["n_cores"], st["blk"], st["nblk"], st["npc"]
    groups, soff = st["groups"], st["soff"]
    tot_slots = st["tot_slots"]
    HD = H * DH                      # 128
    REC = 128                        # fp16 elems per record (256B), all layers
    NROWS = st["NROWS"]
    half_A = st["half_rows"]

    nc = bacc.Bacc(None, target_bir_lowering=False, num_swdge_queues=nq)

    # feats stored pre-transposed per block: [nblk, D_IN, blk] so layer 0
    # needs no on-device transpose.
    feats_own = nc.dram_tensor("feats_own", [nblk, D_IN, blk], F16, kind="ExternalInput")
    idx_in = nc.dram_tensor("idx_mega", [128, 8 * tot_slots], I16, kind="ExternalInput")
    W0_in = nc.dram_tensor("W0", [HD, D_IN], F32, kind="ExternalInput")
    W1_in = nc.dram_tensor("W1", [HD, HD], F32, kind="ExternalInput")
    W2_in = nc.dram_tensor("W2", [D_OUT, HD], F32, kind="ExternalInput")
    M0_in = nc.dram_tensor("M0", [HD, HD + 4], F32, kind="ExternalInput")
    M1_in = nc.dram_tensor("M1", [HD, HD + 4], F32, kind="ExternalInput")
    M2_in = nc.dram_tensor("M2", [D_OUT, D_OUT + 2], F32, kind="ExternalInput")
    RV0_in = nc.dram_tensor("RV0", [1, HD], F32, kind="ExternalInput")
    RV1_in = nc.dram_tensor("RV1", [1, HD], F32, kind="ExternalInput")
    b0_in = nc.dram_tensor("b0", [1, HD], F32, kind="ExternalInput")
    b1_in = nc.dram_tensor("b1", [1, HD], F32, kind="ExternalInput")
    b2_in = nc.dram_tensor("b2", [1, D_OUT], F32, kind="ExternalInput")
    ones_in = nc.dram_tensor("ones", [1, 128], F32, kind="ExternalInput")
    out_t = nc.dram_tensor("out", [npc, D_OUT], F32, kind="ExternalOutput")

    # per layer: feat width, heads, el col offset, wcat cols
    EL0 = H * (DH - 1)                     # 124: els contiguous at 124..127
    LCFG = [
        dict(fw=HD, h=H, elc=EL0, wcat=HD + 4),
        dict(fw=HD, h=H, elc=EL0, wcat=HD + 4),
        dict(fw=D_OUT, h=1, elc=D_OUT, wcat=D_OUT + 2),
    ]

    with tile.TileContext(nc) as tc:
        with tc.tile_pool(name="const", bufs=1) as cpool, \
             tc.tile_pool(name="dram", bufs=1, space="DRAM") as dram, \
             tc.tile_pool(name="xblk", bufs=1) as xpool, \
             tc.tile_pool(name="gat", bufs=5) as gpool, \
             tc.tile_pool(name="et", bufs=5) as epool, \
             tc.tile_pool(name="small", bufs=6) as spool, \
             tc.tile_pool(name="psum", bufs=2, space="PSUM") as ppool, \
             tc.tile_pool(name="psum1", bufs=2, space="PSUM") as ppool2:

            # ---------------- constants / prologue
            idx_sb = cpool.tile([128, 8 * tot_slots], I16)
            nc.sync.dma_start(out=idx_sb[:], in_=idx_in[:])
            ident = cpool.tile([128, 128], F16)
            make_identity(nc, ident[:])
            pad_el = cpool.tile([blk, 8], F16)
            nc.vector.memset(pad_el[:], PAD_EL)

            wins = [W0_in, W1_in, W2_in]
            mins = [M0_in, M1_in, M2_in]
            bins = [b0_in, b1_in, b2_in]
            wcat_sb, brep_sb, wrep_sb = [], [], []
            with tc.tile_pool(name="prolog", bufs=1) as plpool:
                ones_sb = plpool.tile([1, 128], F32)
                nc.sync.dma_start(out=ones_sb[:], in_=ones_in[:])
                for L in range(3):
                    cfg = LCFG[L]
                    kdim = wins[L].shape[0]
                    w_sb = plpool.tile([kdim, wins[L].shape[1]], F32, tag=f"wld{L}", name=f"wld{L}")
                    nc.sync.dma_start(out=w_sb[:], in_=wins[L][:])
                    m_sb = plpool.tile([kdim, cfg["wcat"]], F32, tag=f"mld{L}", name=f"mld{L}")
                    nc.sync.dma_start(out=m_sb[:], in_=mins[L][:])
                    wc_ps = ppool.tile([wins[L].shape[1], cfg["wcat"]], F32, space="PSUM", tag="xtps")
                    nc.tensor.matmul(out=wc_ps[:], lhsT=w_sb[:], rhs=m_sb[:], start=True, stop=True)
                    wc = cpool.tile([wins[L].shape[1], cfg["wcat"]], F16, tag=f"wcat{L}", name=f"wcat{L}")
                    nc.vector.tensor_copy(out=wc[:], in_=wc_ps[:])
                    wcat_sb.append(wc)

                    b_sb = plpool.tile([1, cfg["fw"]], F32, tag=f"bld{L}", name=f"bld{L}")
                    nc.sync.dma_start(out=b_sb[:], in_=bins[L][:])
                    br_ps = ppool.tile([128, cfg["fw"]], F32, space="PSUM", tag="xtps")
                    nc.tensor.matmul(out=br_ps[:], lhsT=ones_sb[:], rhs=b_sb[:], start=True, stop=True)
                    br = cpool.tile([128, cfg["fw"]], F16 if L < 2 else F32,
                                    tag=f"brep{L}", name=f"brep{L}")
                    nc.vector.tensor_copy(out=br[:], in_=br_ps[:])
                    brep_sb.append(br)

                    if L < 2:
                        rv_sb = plpool.tile([1, HD], F32, tag=f"rvld{L}", name=f"rvld{L}")
                        nc.sync.dma_start(out=rv_sb[:], in_=(RV0_in if L == 0 else RV1_in)[:])
                        wr_ps = ppool.tile([128, HD], F32, space="PSUM", tag="xtps")
                        nc.tensor.matmul(out=wr_ps[:], lhsT=ones_sb[:], rhs=rv_sb[:], start=True, stop=True)
                        wr = cpool.tile([128, HD], F16, tag=f"wrep{L}", name=f"wrep{L}")
                        nc.vector.tensor_copy(out=wr[:], in_=wr_ps[:])
                        wrep_sb.append(wr)

            ngrp = len(groups)
            Gs = [b1 - b0 for (b0, b1, _, _) in groups]
            x_tiles = [xpool.tile([128, Gs[gi] * HD], F16, tag=f"x{gi}", name=f"xg{gi}")
                       for gi in range(ngrp)]
            er_tiles = [xpool.tile([128, max(Gs[gi] * H, 1)], F16, tag=f"er{gi}", name=f"erg{gi}")
                        for gi in range(ngrp)]
            rec_tiles = [xpool.tile([128, REC], F16, tag=f"rec{i}", name=f"rect{i}") for i in range(2)]

            slabs = [dram.tile([npc, REC], F16, tag=f"slab{i}", name=f"slab{i}") for i in range(3)]
            tables = [dram.tile([NROWS, REC], F16, tag=f"tab{i}", name=f"tab{i}",
                                addr_space="Shared") for i in range(3)]

            n_real = st["n_real"]
            gq = [0]  # round-robin queue counter
            x0_cache = [None]  # current L0 group feats tile

            def block_group(b):
                for gi, (b0, b1, _, _) in enumerate(groups):
                    if b0 <= b < b1:
                        return gi, b - b0
                raise AssertionError

            def dense_block(L, b):
                cfg = LCFG[L]
                gi, g = block_group(b)
                if L == 0:
                    if g == 0:
                        G = groups[gi][1] - groups[gi][0]
                        x0g = spool.tile([128, G * D_IN], F16, tag="x0g", name="x0g")
                        nc.sync.dma_start(
                            out=x0g[:].rearrange("d (g p) -> d g p", g=G),
                            in_=feats_own[groups[gi][0]:groups[gi][1], :, :]
                                .rearrange("g d p -> d g p"))
                        x0_cache[0] = x0g
                    lhs_ap = x0_cache[0][:, g * D_IN:(g + 1) * D_IN]
                else:
                    x_sb = x_tiles[gi][:, g * HD:(g + 1) * HD]
                    xt_ps = ppool.tile([128, 128], F16, space="PSUM", tag="xtps", name="xt_ps")
                    nc.tensor.transpose(out=xt_ps[:], in_=x_sb, identity=ident[:])
                    xt_sb = spool.tile([128, 128], F16, tag="xts", name="xt_sb")
                    if b % 2 == 0:
                        nc.scalar.copy(out=xt_sb[:], in_=xt_ps[:])
                    else:
                        nc.vector.tensor_copy(out=xt_sb[:], in_=xt_ps[:])
                    lhs_ap = xt_sb[:]
                y_ps = ppool2.tile([128, cfg["wcat"]], F32, space="PSUM", tag="yps", name="y_ps")
                nc.tensor.matmul(out=y_ps[:], lhsT=lhs_ap, rhs=wcat_sb[L][:], start=True, stop=True)
                recb = rec_tiles[b % 2]
                ncopy = cfg["fw"] + (1 if L == 2 else 0)   # L2: feat + el col
                nc.scalar.copy(out=recb[:, 0:ncopy], in_=y_ps[:, 0:ncopy])
                erc = cfg["fw"] + (1 if L == 2 else 0)
                h = cfg["h"]
                nc.vector.tensor_copy(out=er_tiles[gi][:, g * h:(g + 1) * h],
                                      in_=y_ps[:, erc:erc + h])
                nc.sync.dma_start(out=slabs[L][b * blk:(b + 1) * blk, :],
                                  in_=recb[:, 0:REC])

            def poison_pads(L):
                cfg = LCFG[L]
                npad = npc - n_real
                nc.sync.dma_start(
                    out=slabs[L][n_real:npc, cfg["elc"]:cfg["elc"] + cfg["h"]],
                    in_=pad_el[0:npad, 0:cfg["h"]])

            def allgather_chunk(L, b0, b1):
                out_v = tables[L][:].rearrange("(c r) e -> c r e", c=n_cores)
                nc.gpsimd.collective_compute(
                    "AllGather", mybir.AluOpType.bypass,
                    replica_groups=[list(range(n_cores))],
                    ins=[slabs[L][b0 * blk:b1 * blk, :].opt()],
                    outs=[out_v[:, b0 * blk:b1 * blk, :]],
                )

            def gather_half(grid, L, gi, half, dst0, S):
                """Chunked gathers for a (group, half) slot run.

                16 slots (2048 descriptors, 8KB of Q7 idx scratch) per
                instruction — larger chunks overflow the Q7 scratch and run
                ~3x slower per descriptor.
                """
                o = soff[(gi, half)]
                tab = tables[L][0:half_A, :] if half == 0 else tables[L][half_A:NROWS, :]
                for t0 in range(0, S, 8):
                    tw = min(8, S - t0)
                    q = gq[0] % nq
                    gq[0] += 1
                    nc.gpsimd.dma_gather(
                        out_ap=grid[:, dst0 + t0:dst0 + t0 + tw, :],
                        in_ap=tab,
                        idxs_ap=idx_sb[:, (o + t0) * 8:(o + t0 + tw) * 8],
                        num_idxs=128 * tw,
                        num_idxs_reg=128 * tw,
                        elem_size=REC,
                        single_packet=True,
                        queue_num=q,
                    )

            def el_view3(L, ap, G, Tg):
                """[128, Tg, G, h] el view of a t-major (group-half) run.

                L<2 record layout: 4x31 kept feats at 0..123, els contiguous
                at 124..127; L2: 64 feats then el at col 64."""
                cfg = LCFG[L]
                e0 = cfg["elc"]
                return ap[:, :, e0:e0 + cfg["h"]].rearrange(
                    "p (t g) e -> p t g e", g=G)

            def edge_group(L, gi):
                cfg = LCFG[L]
                h, fw = cfg["h"], cfg["fw"]
                dh = fw // h
                b0, b1, TAg, TBg = groups[gi]
                G = b1 - b0
                SA, SB = G * TAg, G * TBg
                S = SA + SB
                assert SA > 0 and SB > 0

                grid = gpool.tile([128, S, REC], F16, tag="grid")
                gather_half(grid, L, gi, 0, 0, SA)
                gather_half(grid, L, gi, 1, SA, SB)

                et = epool.tile([128, S, h], F16, tag="et")
                erv = er_tiles[gi][:, 0:G * h].rearrange("p (g h) -> p g h", g=G)
                # fused: et = el + er  (pad slots carry el = -30000 from the
                # poisoned pad row, so they stay ~-30000 and exp to 0)
                for (s0, Sh, Tg) in ((0, SA, TAg), (SA, SB, TBg)):
                    nc.vector.tensor_tensor(
                        out=et[:, s0:s0 + Sh, :].rearrange("p (t g) h -> p t g h", g=G),
                        in0=el_view3(L, grid[:, s0:s0 + Sh, 0:REC], G, Tg),
                        in1=erv.unsqueeze(1).to_broadcast([128, Tg, G, h]),
                        op=mybir.AluOpType.add)
                # leaky relu on the whole tile (flat 2D -> DVE fast mode)
                nc.vector.scalar_tensor_tensor(
                    out=et[:], in0=et[:], scalar=0.2, in1=et[:],
                    op0=mybir.AluOpType.mult, op1=mybir.AluOpType.max)
                # per-(node,head) max over slots: A and B reduced separately
                mxa = spool.tile([128, G * h], F16, tag="mxa")
                mxb = spool.tile([128, G * h], F16, tag="mxb")
                with nc.allow_low_precision(reason="fp16 softmax stats: bounded terms"):
                    nc.vector.tensor_reduce(
                        out=mxa[:].rearrange("p (g h) -> p g h", g=G),
                        in_=et[:, 0:SA, :].rearrange("p (t g) h -> p g h t", g=G),
                        axis=mybir.AxisListType.X, op=mybir.AluOpType.max)
                    nc.vector.tensor_reduce(
                        out=mxb[:].rearrange("p (g h) -> p g h", g=G),
                        in_=et[:, SA:S, :].rearrange("p (t g) h -> p g h t", g=G),
                        axis=mybir.AxisListType.X, op=mybir.AluOpType.max)
                nc.vector.tensor_tensor(out=mxa[:], in0=mxa[:], in1=mxb[:],
                                        op=mybir.AluOpType.max)
                for (s0, Sh, Tg) in ((0, SA, TAg), (SA, SB, TBg)):
                    etv = et[:, s0:s0 + Sh, :].rearrange("p (t g) h -> p t (g h)", g=G)
                    nc.vector.tensor_tensor(
                        out=etv, in0=etv,
                        in1=mxa[:].unsqueeze(1).to_broadcast([128, Tg, G * h]),
                        op=mybir.AluOpType.subtract)
                nc.scalar.activation(out=et[:], in_=et[:],
                                     func=mybir.ActivationFunctionType.Exp)
                dena = spool.tile([128, G * h], F16, tag="dena")
                denb = spool.tile([128, G * h], F16, tag="denb")
                with nc.allow_low_precision(reason="fp16 softmax stats: bounded terms"):
                    nc.vector.tensor_reduce(
                        out=dena[:].rearrange("p (g h) -> p g h", g=G),
                        in_=et[:, 0:SA, :].rearrange("p (t g) h -> p g h t", g=G),
                        axis=mybir.AxisListType.X, op=mybir.AluOpType.add)
                    nc.vector.tensor_reduce(
                        out=denb[:].rearrange("p (g h) -> p g h", g=G),
                        in_=et[:, SA:S, :].rearrange("p (t g) h -> p g h t", g=G),
                        axis=mybir.AxisListType.X, op=mybir.AluOpType.add)
                    nc.vector.tensor_tensor(out=dena[:], in0=dena[:], in1=denb[:],
                                            op=mybir.AluOpType.add)
                    rden = spool.tile([128, G * h], F16, tag="rden")
                    nc.vector.reciprocal(out=rden[:], in_=dena[:])
                # grid *= exp (unnormalized; 1/den applied to the aggregate)
                for (s0, Sh) in ((0, SA), (SA, SB)):
                    if L < 2:
                        featm = grid[:, s0:s0 + Sh, 0:EL0].rearrange(
                            "p s (h d) -> p s h d", h=h)
                        exb = et[:, s0:s0 + Sh, :].unsqueeze(3).to_broadcast(
                            [128, Sh, h, DH - 1])
                        nc.vector.tensor_tensor(out=featm, in0=featm, in1=exb,
                                                op=mybir.AluOpType.mult)
                        elp = grid[:, s0:s0 + Sh, EL0:EL0 + h]
                        nc.vector.tensor_tensor(out=elp, in0=elp,
                                                in1=et[:, s0:s0 + Sh, :],
                                                op=mybir.AluOpType.mult)
                    else:
                        featm = grid[:, s0:s0 + Sh, 0:fw]
                        exb = et[:, s0:s0 + Sh, :].to_broadcast([128, Sh, fw])
                        nc.vector.tensor_tensor(out=featm, in0=featm, in1=exb,
                                                op=mybir.AluOpType.mult)
                # t-major tree-reduce: every level is one flat contiguous add
                for (s0, Tg) in ((0, TAg), (SA, TBg)):
                    Tr = Tg
                    while Tr > 1:
                        hlf = Tr // 2
                        dsts = grid[:, s0:s0 + hlf * G, 0:fw]
                        srcs = grid[:, s0 + (Tr - hlf) * G:s0 + Tr * G, 0:fw]
                        nc.vector.tensor_tensor(out=dsts, in0=dsts, in1=srcs,
                                                op=mybir.AluOpType.add)
                        Tr -= hlf
                agg = grid[:, 0:G, 0:fw]                       # [128, G, fw]
                nc.vector.tensor_tensor(out=agg, in0=agg, in1=grid[:, SA:SA + G, 0:fw],
                                        op=mybir.AluOpType.add)
                rdv = rden[:].rearrange("p (g h) -> p g h", g=G)
                if L < 2:
                    # softmax normalization on the aggregate (kept feats + el)
                    aggm = agg[:, :, 0:EL0].rearrange("p g (h d) -> p g h d", h=h)
                    nc.vector.tensor_tensor(
                        out=aggm, in0=aggm,
                        in1=rdv.unsqueeze(3).to_broadcast([128, G, h, DH - 1]),
                        op=mybir.AluOpType.mult)
                    agge = agg[:, :, EL0:EL0 + h]
                    nc.vector.tensor_tensor(out=agge, in0=agge, in1=rdv,
                                            op=mybir.AluOpType.mult)
                    # reconstruct the displaced feat coordinate per head
                    t31 = spool.tile([128, G * fw], F16, tag="t31")
                    t31v = t31[:].rearrange("p (g f) -> p g f", g=G)
                    nc.vector.tensor_tensor(
                        out=t31v, in0=agg,
                        in1=wrep_sb[L][:].unsqueeze(1).to_broadcast([128, G, fw]),
                        op=mybir.AluOpType.mult)
                    x31 = spool.tile([128, G * h], F16, tag="x31")
                    x31v = x31[:].rearrange("p (g h) -> p g h", g=G)
                    with nc.allow_low_precision(reason="fp16 32-term reconstruction dot"):
                        nc.vector.tensor_reduce(
                            out=x31v,
                            in_=t31v[:, :, 0:EL0].rearrange(
                                "p g (h d) -> p g h d", h=h),
                            axis=mybir.AxisListType.X, op=mybir.AluOpType.add)
                        # write the reconstructed coordinate straight into the
                        # aggregate's el slots (fuses the writeback copy)
                        nc.vector.tensor_tensor(
                            out=agg[:, :, EL0:EL0 + h], in0=x31v,
                            in1=t31v[:, :, EL0:EL0 + h],
                            op=mybir.AluOpType.add)
                    nc.vector.tensor_tensor(
                        out=agg, in0=agg,
                        in1=brep_sb[L][:].unsqueeze(1).to_broadcast([128, G, fw]),
                        op=mybir.AluOpType.add)
                    nc.scalar.activation(
                        out=x_tiles[gi][:].rearrange("p (g f) -> p g f", g=G),
                        in_=agg, func=mybir.ActivationFunctionType.Relu)
                    if b1 == nblk and npc > n_real:
                        # pad rows reconstruct to +-inf; zero them so the next
                        # dense matmul stays finite. predicate (base - p >= 0)
                        # keeps real partitions, fills pads with 0.
                        xl = x_tiles[gi][:, (G - 1) * HD:G * HD]
                        nc.gpsimd.affine_select(
                            out=xl, in_=xl,
                            compare_op=mybir.AluOpType.is_ge,
                            fill=0.0,
                            base=n_real - (nblk - 1) * blk - 1,
                            pattern=[[0, HD]],
                            channel_multiplier=-1,
                        )
                else:
                    nc.vector.tensor_tensor(
                        out=agg, in0=agg,
                        in1=rdv.to_broadcast([128, G, fw]),
                        op=mybir.AluOpType.mult)
                    ob = spool.tile([128, G * D_OUT], F32, tag="ob")
                    obv = ob[:].rearrange("p (g f) -> p g f", g=G)
                    nc.scalar.copy(out=obv, in_=agg)
                    nc.vector.tensor_tensor(
                        out=obv, in0=obv,
                        in1=brep_sb[2][:].unsqueeze(1).to_broadcast([128, G, D_OUT]),
                        op=mybir.AluOpType.add)
                    nc.sync.dma_start(
                        out=out_t[b0 * blk:b1 * blk, :].rearrange("(g p) d -> p g d", g=G),
                        in_=obv)

            for b in range(nblk):
                dense_block(0, b)
            poison_pads(0)
            allgather_chunk(0, 0, nblk)

            for L in range(3):
                for gi in range(ngrp):
                    edge_group(L, gi)
                    if L < 2:
                        b0, b1, _, _ = groups[gi]
                        for b in range(b0, b1):
                            dense_block(L + 1, b)
                if L < 2:
                    poison_pads(L + 1)
                    allgather_chunk(L + 1, 0, nblk)

    nc.compile()
    return nc


# ------------------------------------------------------------------- runner

def make_in_maps(st, inputs, H=4, DH=32, D_IN=128, D_OUT=64):
    n_cores, npc = st["n_cores"], st["npc"]
    feats = np.asarray(inputs["feats"], np.float32)
    HD = H * DH

    def mk_basis(al, ar):
        """M' [HD, HD+4] mapping y=Wx -> [rec | er] with record layout
        [head0 31 feats | head1 31 feats | ... | el0..el3 at 124..127], the
        reconstruction coefficient vector rv [HD] (aligned to the record:
        cols 0..123 weight the kept feats, 124..127 weight the el slots),
        and perm such that x_rec[c] = x_feat[perm[c]] (where the el slot c
        holds the reconstructed displaced coordinate)."""
        h, dh = al.shape
        kept = dh - 1                      # 31 kept coords per head
        M = np.zeros((HD, HD + h), np.float32)
        rv = np.zeros(HD, np.float32)
        perm = np.zeros(HD, np.int64)
        for hh in range(h):
            base = hh * dh
            j = int(np.argmax(np.abs(al[hh])))
            sig = np.arange(dh)
            sig[j], sig[dh - 1] = sig[dh - 1], sig[j]
            for d_ in range(kept):
                M[base + sig[d_], hh * kept + d_] = 1.0
                perm[hh * kept + d_] = base + sig[d_]
                rv[hh * kept + d_] = -al[hh][sig[d_]] / al[hh][j]
            M[base:base + dh, h * kept + hh] = al[hh]
            M[base:base + dh, HD + hh] = ar[hh]
            perm[h * kept + hh] = base + j
            rv[h * kept + hh] = 1.0 / al[hh][j]
        return M, rv, perm

    al0 = np.asarray(inputs["al0"]); ar0 = np.asarray(inputs["ar0"])
    al1 = np.asarray(inputs["al1"]); ar1 = np.asarray(inputs["ar1"])
    al2 = np.asarray(inputs["al2"]); ar2 = np.asarray(inputs["ar2"])

    M0, rv0, perm0 = mk_basis(al0, ar0)
    M1, rv1, perm1 = mk_basis(al1, ar1)

    M2 = np.zeros((D_OUT, D_OUT + 2), np.float32)
    M2[:, :D_OUT] = np.eye(D_OUT, dtype=np.float32)
    M2[:, D_OUT] = al2[0]
    M2[:, D_OUT + 1] = ar2[0]

    W0 = np.asarray(inputs["W0"], np.float32)
    W1 = np.asarray(inputs["W1"], np.float32)[:, perm0]
    W2 = np.asarray(inputs["W2"], np.float32)[:, perm1]
    b0 = np.asarray(inputs["b0"], np.float32)[perm0]
    b1 = np.asarray(inputs["b1"], np.float32)[perm1]

    shared = dict(
        W0=W0, W1=W1, W2=W2, M0=M0, M1=M1, M2=M2,
        RV0=rv0.reshape(1, -1), RV1=rv1.reshape(1, -1),
        b0=b0.reshape(1, -1), b1=b1.reshape(1, -1),
        b2=np.asarray(inputs["b2"], np.float32).reshape(1, -1),
        ones=np.ones((1, 128), np.float32),
    )
    nblk, blk = st["nblk"], st["blk"]
    in_maps = []
    for c in range(n_cores):
        lay = st["layouts"][c]
        fo = np.zeros((npc, D_IN), np.float16)
        valid = lay >= 0
        fo[valid] = feats[lay[valid]].astype(np.float16)
        # pre-transposed per block: [nblk, D_IN, blk]
        fo = np.ascontiguousarray(
            fo.reshape(nblk, blk, D_IN).transpose(0, 2, 1))
        m = dict(shared)
        m["feats_own"] = fo
        m["idx_mega"] = st["idx_mega"][c]
        in_maps.append(m)
    return in_maps


def assemble_output(st, results, N, D_OUT=64):
    out = np.zeros((N, D_OUT), np.float32)
    for c, r in enumerate(results):
        lay = st["layouts"][c]
        valid = lay >= 0
        out[lay[valid]] = r["out"][valid]
    return out


# =================================================================== kernel

_CACHE = {}
LAST_EXEC_NS = None
LAST_TRACE = None
LAST_RES = None


def kernel(**inputs):
    """Full-input GAT forward on 8 NeuronCores; returns [50000, 64] f32."""
    global LAST_EXEC_NS, LAST_TRACE, LAST_RES
    from concourse.bass_utils import run_bass_kernel_spmd

    N = 50000
    src = np.asarray(inputs["src"])
    dst = np.asarray(inputs["dst"])

    if "prog" not in _CACHE:
        st = prep_host(src, dst, N, n_cores=8)
        nc = build_program(st)
        _CACHE["prog"] = (st, nc)
    st, nc = _CACHE["prog"]

    in_maps = make_in_maps(st, inputs)
    trace = os.environ.get("GAT_TRACE", "0") == "1"
    res = run_bass_kernel_spmd(nc, in_maps, core_ids=list(range(8)), trace=trace)
    LAST_EXEC_NS = res.exec_time_ns
    LAST_RES = res
    if res.instructions_and_trace:
        LAST_TRACE = res.instructions_and_trace[1]
    return assemble_output(st, [res.results[c] for c in range(8)], N)
